# revision 49
# baseline (speedup 1.0000x reference)
"""JointAttention TRN2 Bass kernel.

Sharding: 8 cores = batch(2) x head-group(4). Each core owns one batch
element and 4 of the 16 heads (a 256-wide channel slice of every
projection). Partial outputs are summed on the host (row-parallel Wo).

Numerics / performance design (per core):
  - Projections run in bf16 (1 cyc/row): proj noise ~0.4%, which matters
    because softmax-weight noise transfers ~1:1 into the attention output.
  - q/k are stored as fp8e4m3 hi + lo residual pairs arranged so ONE
    DoubleRow matmul (0.5 cyc/row, 256-deep contraction) computes the
    EXACT (hi+lo)x(hi+lo) score: per head, the k tile stacks [k_hi;k_lo]
    along partitions (replicated over the pair dim), the q tile holds
    q_hi/q_lo in the pair dim (replicated along partitions).
  - exp splits across both elementwise engines: ACT computes exact
    exp(s/8 - 2ln2) into bf16; DVE computes a Schraudolph exp - one
    fp32->uint8 convert (HW-verified: saturating, round-nearest-even)
    whose result IS the fp8e4m3 bit pattern of ~exp(s)/4.
  - PV: ACT chunks use bf16 matmuls (exact p path), DVE chunks use
    fp8 DoubleRow with V hi + lo residual matmuls. A ones-column gives
    the softmax denominator in PSUM row 64.
  - PV matmuls trail their exp by PV_LAG units and the softmax division
    (reciprocal + broadcast-DMA + one multiply) trails by one head, so
    the in-order PE/DVE queues never stall the exp pipeline.
  - Output projection in fp32r; out-proj for query block qc is emitted
    during qc+1 so its PSUM drain overlaps the exp stream.
"""

import sys

import numpy as np

if "/opt/trn_rl_repo" not in sys.path:
    sys.path.insert(0, "/opt/trn_rl_repo")

import ml_dtypes
import concourse.bass as bass
import concourse.tile as tile
from concourse import bacc, mybir
from concourse.bass_utils import run_bass_kernel_spmd

F32 = mybir.dt.float32
F32R = mybir.dt.float32r
BF16 = mybir.dt.bfloat16
FP8 = mybir.dt.float8e4
U8 = mybir.dt.uint8
AFT = mybir.ActivationFunctionType
ALU = mybir.AluOpType
DR = mybir.MatmulPerfMode.DoubleRow

NP_BF16 = ml_dtypes.bfloat16

D = 1024          # model dim
T = 2048          # query length (= self key length)
TK = 4096         # total key length (self + context)
CS = 256          # channels per core (4 heads x 64)
NH = 4            # heads per core
HD = 64           # head dim
N_CORES = 8

# exp paths (both produce exp(s_true) * 2^-4):
#   ACT: exp(raw * 0.125 - 4ln2) -> bf16
#   DVE: uint8(raw * (8/ln2)/8 + 24) viewed as fp8e4m3 bits
# (B=24 keeps Schraudolph bits <= ~116 for the data's max score 7.96,
#  clear of the fp8 inf/NaN encodings at bits >= 120.)
# raw score = sum_64 q*k ~ N(0, 8^2); s_true = raw/8.
EXP_ACT_SCALE = 0.125
EXP_ACT_BIAS = -2.772588722239781    # -4 ln 2
EXP_DVE_MULT = 1.4426950408889634    # (8/ln2) / 8
EXP_DVE_BIAS = 24.0
# fraction of exp chunks handled by the scalar engine (rest on vector)
ACT_FRAC = 0.57
# how many exp units a PV matmul trails its exp by (PE queue decoupling)
PV_LAG = 5

VSLOT8 = 144      # vt8 per-head slot: v_hi[0:64], ones at 64, v_lo[72:136]
VSLOTB = 80       # vtb per-head slot: v[0:64], ones at 64


def build_nc(with_vbias: bool):
    nc = bacc.Bacc(None)

    xT = nc.declare_dram_parameter("xT", [D, T], BF16, isOutput=False)
    cT = nc.declare_dram_parameter("cT", [D, T], BF16, isOutput=False)
    wq = nc.declare_dram_parameter("wq", [D, CS], BF16, isOutput=False)
    wks = nc.declare_dram_parameter("wks", [D, CS], BF16, isOutput=False)
    wkc = nc.declare_dram_parameter("wkc", [D, CS], BF16, isOutput=False)
    wvs = nc.declare_dram_parameter("wvs", [D, CS], BF16, isOutput=False)
    wvc = nc.declare_dram_parameter("wvc", [D, CS], BF16, isOutput=False)
    bqk = nc.declare_dram_parameter("bqk", [128, 6], F32, isOutput=False)
    bv = nc.declare_dram_parameter("bv", [2, CS], F32, isOutput=False)
    wo = nc.declare_dram_parameter("wo", [CS, D], F32, isOutput=False)
    out = nc.declare_dram_parameter("out", [T, D], F32, isOutput=True)

    with tile.TileContext(nc) as tc:
        _emit(nc, tc, xT, cT, wq, wks, wkc, wvs, wvc, bqk, bv, wo, out,
              with_vbias)
    nc.compile()
    return nc


def _emit(nc, tc, xT, cT, wq, wks, wkc, wvs, wvc, bqk, bv, wo, out,
          with_vbias):
    from contextlib import ExitStack

    ctx = ExitStack()
    with ctx:
        consts = ctx.enter_context(tc.tile_pool(name="consts", bufs=1))
        io_pool = ctx.enter_context(tc.tile_pool(name="io", bufs=5))
        qk_pool = ctx.enter_context(tc.tile_pool(name="qk", bufs=1))
        v_pool = ctx.enter_context(tc.tile_pool(name="v", bufs=1))
        ptb_pool = ctx.enter_context(tc.tile_pool(name="ptb", bufs=6))
        pt8_pool = ctx.enter_context(tc.tile_pool(name="pt8", bufs=6))
        outt_pool = ctx.enter_context(tc.tile_pool(name="outt", bufs=1))
        dn_pool = ctx.enter_context(tc.tile_pool(name="dn", bufs=2))
        odd_pool = ctx.enter_context(tc.tile_pool(name="odd", bufs=2))
        st_pool = ctx.enter_context(tc.tile_pool(name="st", bufs=2))
        ps_s = ctx.enter_context(
            tc.tile_pool(name="ps_s", bufs=3, space="PSUM"))
        ps_pv = ctx.enter_context(
            tc.tile_pool(name="ps_pv", bufs=2, space="PSUM"))

        # ---- constants (first io tiles prefetched ahead of the bulky
        # weight loads so the first projection matmuls start ASAP; wo is
        # loaded last - it is first needed at the qc0 out-projection) ----
        io_prefetch = {}

        def fetch_io(j, split=False):
            t = io_pool.tile([128, 8, 512], BF16, tag="io", name=f"io_0_{j}")
            srcv = xT.rearrange("(a p) t -> p a t", p=128)
            if split:
                nc.sync.dma_start(out=t[:, 0:4, :],
                                  in_=srcv[:, 0:4, j * 512:(j + 1) * 512])
                nc.sync.dma_start(out=t[:, 4:8, :],
                                  in_=srcv[:, 4:8, j * 512:(j + 1) * 512])
            else:
                nc.sync.dma_start(out=t,
                                  in_=srcv[:, :, j * 512:(j + 1) * 512])
            io_prefetch[(0, j)] = t

        w_sb = {}

        def fetch_w(name, w):
            t = consts.tile([128, 8, CS], BF16, tag=f"w_{name}",
                            name=f"w_{name}")
            nc.sync.dma_start(out=t, in_=w.rearrange("(a p) c -> p a c", p=128))
            w_sb[name] = t

        fetch_io(0, split=True)
        fetch_w("wq", wq)
        fetch_w("wks", wks)
        fetch_io(1)
        fetch_w("wvs", wvs)
        fetch_w("wkc", wkc)
        fetch_w("wvc", wvc)
        bqk_sb = consts.tile([128, 6], F32, tag="bqk", name="bqk")
        nc.sync.dma_start(out=bqk_sb, in_=bqk[:, :])
        ebias_sb = consts.tile([128, 1], F32, tag="ebias", name="ebias")
        nc.gpsimd.memset(ebias_sb[:, :], EXP_ACT_BIAS)
        wo_sb = consts.tile([128, 2, D], F32R, tag="wo", name="wo")
        nc.sync.dma_start(out=wo_sb,
                          in_=wo.rearrange("(a p) f -> p a f", p=128).bitcast(F32R))
        if with_vbias:
            bv_sb = consts.tile([128, 2, CS], F32R, tag="bv", name="bv")
            nc.sync.dma_start(out=bv_sb[0:1, :, :],
                              in_=bv[:, :].rearrange("s c -> 1 s c").bitcast(F32R))
            ones_sb = consts.tile([128, 128], F32R, tag="ones", name="ones")
            nc.vector.memset(ones_sb[0:1, :], 1.0)

        # ---- persistent activation tiles ----
        # Per head h: qx[h][p, i, t]: i=0 -> q_hi[ch p%64], i=1 -> q_lo
        #             kx[h][p, i, t]: rows 0:64 k_hi, 64:128 k_lo (both i)
        qx = [qk_pool.tile([128, 2, T], FP8, tag=f"qx{h}", name=f"qx{h}")
              for h in range(NH)]
        kx = [qk_pool.tile([128, 2, TK], FP8, tag=f"kx{h}", name=f"kx{h}")
              for h in range(NH)]
        vt8 = [v_pool.tile([128, 2, NH * VSLOT8], FP8, tag=f"vt8_{kc}",
                           name=f"vt8_{kc}") for kc in range(16)]
        vtb = [v_pool.tile([128, 2, NH * VSLOTB], BF16, tag=f"vtb_{kc}",
                           name=f"vtb_{kc}") for kc in range(16)]
        for kc in range(16):
            nc.gpsimd.memset(vt8[kc][:, :, :].rearrange(
                "p i (h w) -> p i h w", h=NH)[:, :, :, 64:65], 1.0)
            nc.gpsimd.memset(vtb[kc][:, :, :].rearrange(
                "p i (h w) -> p i h w", h=NH)[:, :, :, 64:65], 1.0)
        outT = [outt_pool.tile([128, T], F32R, tag=f"outT{cc}",
                               name=f"outT{cc}") for cc in range(2)]

        # ---- projections (bf16) ----
        def emit_proj_block(src_i, tc4):
            src, wk_n, wv_n = ((xT, "wks", "wvs"), (cT, "wkc", "wvc"))[src_i]
            if True:
                ts = slice(tc4 * 512, (tc4 + 1) * 512)
                if (src_i, tc4) in io_prefetch:
                    io = io_prefetch[(src_i, tc4)]
                else:
                    io = io_pool.tile([128, 8, 512], BF16, tag="io",
                                      name=f"io_{src_i}_{tc4}")
                    nc.sync.dma_start(
                        out=io,
                        in_=src.rearrange("(a p) t -> p a t", p=128)[:, :, ts])

                projs = [(wk_n, 1, kx, src_i * T)]
                if src_i == 0:
                    projs.append(("wq", 0, qx, 0))
                for wn, brow, dst, coff in projs:
                    dsl = slice(coff + tc4 * 512, coff + (tc4 + 1) * 512)
                    ps = ps_s.tile([128, 1024], F32, tag="s",
                                   name=f"ps_{wn}_{src_i}_{tc4}")
                    for cc in range(2):
                        hs = slice(cc * 512, (cc + 1) * 512)
                        csl = slice(cc * 128, (cc + 1) * 128)
                        for kk in range(8):
                            nc.tensor.matmul(
                                ps[:, hs], w_sb[wn][:, kk, csl],
                                io[:, kk, :],
                                start=(kk == 0), stop=(kk == 7))
                    for cc in range(2):
                        hs = slice(cc * 512, (cc + 1) * 512)
                        for par in range(2):  # 0: head 2cc, 1: head 2cc+1
                            hh = 2 * cc + par
                            rows = slice(par * 64, par * 64 + 64)
                            bias = bqk_sb[rows, brow * 2 + cc:
                                          brow * 2 + cc + 1]
                            dt = dst[hh]
                            with nc.allow_low_precision(
                                    reason="fp8 hi/lo storage"):
                                # hi and lo land in the partition-aligned
                                # half of the destination tile; DMAs below
                                # rearrange to the final layout.
                                nc.scalar.activation(
                                    dt[rows, 0, dsl], ps[rows, hs],
                                    AFT.Identity, bias=bias, scale=1.0)
                                nc.vector.scalar_tensor_tensor(
                                    dt[rows, 1, dsl], ps[rows, hs],
                                    1.0, dt[rows, 0, dsl],
                                    ALU.mult, ALU.subtract)
                            if dst is qx:
                                # final layout: (hi, lo) duplicated across
                                # both partition halves
                                other = slice(64 - par * 64,
                                              128 - par * 64)
                                nc.sync.dma_start(out=dt[other, :, dsl],
                                                  in_=dt[rows, :, dsl])
                            else:
                                # final layout: rows 0:64 = hi, 64:128 =
                                # lo, both replicated along i
                                if par == 0:
                                    # wrote hi->[0:64,0], lo->[0:64,1]
                                    nc.sync.dma_start(
                                        out=dt[64:128, 0, dsl],
                                        in_=dt[0:64, 1, dsl])
                                    nc.sync.dma_start(
                                        out=dt[64:128, 1, dsl],
                                        in_=dt[0:64, 1, dsl])
                                    nc.sync.dma_start(
                                        out=dt[0:64, 1, dsl],
                                        in_=dt[0:64, 0, dsl])
                                else:
                                    # wrote hi->[64:128,0], lo->[64:128,1]
                                    nc.sync.dma_start(
                                        out=dt[0:64, 0, dsl],
                                        in_=dt[64:128, 0, dsl])
                                    nc.sync.dma_start(
                                        out=dt[0:64, 1, dsl],
                                        in_=dt[64:128, 0, dsl])
                                    nc.sync.dma_start(
                                        out=dt[64:128, 0, dsl],
                                        in_=dt[64:128, 1, dsl])

                # ---- V projection (psums borrow the pv-pool banks,
                # idle during projections, so the q/k "s" ring never
                # blocks the PE on psum drains) ----
                for sub in range(4):
                    pvt = ps_pv.tile([128, 512], F32, tag="pv",
                                     name=f"ps_v_{src_i}_{tc4}_{sub}")
                    for kk in range(8):
                        nc.tensor.matmul(
                            pvt[:, 0:CS],
                            io[:, kk, sub * 128:(sub + 1) * 128],
                            w_sb[wv_n][:, kk, :],
                            start=(kk == 0),
                            stop=(kk == 7 and not with_vbias))
                    if with_vbias:
                        nc.tensor.matmul(
                            pvt[:, 0:CS], ones_sb[0:1, :],
                            bv_sb[0:1, src_i, :],
                            start=False, stop=True)
                    gchunk = src_i * 16 + tc4 * 4 + sub
                    kc256, i = gchunk // 2, gchunk % 2
                    psv = pvt[:, 0:CS].rearrange(
                        "p (h w) -> p h w", h=NH)
                    v8 = vt8[kc256][:, i, :].rearrange(
                        "p (h w) -> p h w", h=NH)
                    vb = vtb[kc256][:, i, :].rearrange(
                        "p (h w) -> p h w", h=NH)
                    with nc.allow_low_precision(reason="fp8/bf16 V"):
                        nc.scalar.activation(
                            v8[:, :, 0:64], psv, AFT.Copy)
                        nc.scalar.activation(
                            vb[:, :, 0:64], psv, AFT.Copy)
                        nc.vector.scalar_tensor_tensor(
                            v8[:, :, 72:136], psv, 1.0,
                            v8[:, :, 0:64], ALU.mult, ALU.subtract)

        for _src_i in range(2):
            for _tc4 in range(4):
                emit_proj_block(_src_i, _tc4)

        # ---- attention ----
        unit = 0
        n_act = 0

        def oproj_pieces(qc, tail=False):
            def piece(qt):
                def emit():
                    qsl = slice(qc * 512 + qt * 128,
                                qc * 512 + (qt + 1) * 128)
                    o_ps = ps_s.tile([128, 1024], F32, tag="s",
                                     name=f"o_{qc}_{qt}")
                    for fc in range(2):
                        fsl = slice(fc * 512, (fc + 1) * 512)
                        for cc in range(2):
                            nc.tensor.matmul(
                                o_ps[:, fsl], outT[cc][:, qsl],
                                wo_sb[:, cc, fsl],
                                start=(cc == 0), stop=(cc == 1))
                    st = st_pool.tile([128, 1024], F32, tag="st",
                                      name=f"st_{qc}_{qt}")
                    if tail:
                        # split drains so copy/DMA latency chains overlap
                        for fc in range(2):
                            fsl = slice(fc * 512, (fc + 1) * 512)
                            if fc == 0:
                                nc.vector.tensor_copy(st[:, fsl],
                                                      o_ps[:, fsl])
                            else:
                                nc.scalar.activation(st[:, fsl],
                                                     o_ps[:, fsl], AFT.Copy)
                            nc.sync.dma_start(out=out[qsl, fsl],
                                              in_=st[:, fsl])
                        return
                    if qt % 2 == 0:
                        nc.vector.tensor_copy(st, o_ps)
                    else:
                        nc.scalar.activation(st, o_ps, AFT.Copy)
                    nc.sync.dma_start(out=out[qsl, :], in_=st)
                return emit
            return [piece(qt) for qt in range(4)]

        pending_div = None
        pending_oproj = []
        for qc in range(4):
            qs = slice(qc * 512, (qc + 1) * 512)
            for h in range(4):
                if qc > 0 and h == 1:
                    pending_oproj.extend(oproj_pieces(qc - 1))
                pv = ps_pv.tile([128, 512], F32, tag="pv", name=f"pv_{qc}_{h}")
                pending_pv = []
                for kc in range(16):
                    s_ps = ps_s.tile([128, 1024], F32, tag="s",
                                     name=f"s_{qc}_{h}_{kc}")
                    for i in range(2):
                        nc.tensor.matmul(
                            s_ps[:, i * 512:(i + 1) * 512],
                            kx[h][:, :, kc * 256 + i * 128:
                                  kc * 256 + (i + 1) * 128],
                            qx[h][:, :, qs],
                            start=True, stop=True, perf_mode=DR)
                    sv = s_ps[:, :].rearrange("p (i q) -> p i q", i=2)
                    want_act = int((unit + 1) * ACT_FRAC) > n_act
                    unit += 1
                    if want_act:
                        n_act += 1
                        pt = ptb_pool.tile([128, 2, 512], BF16, tag="ptb",
                                           name=f"pt_{qc}_{h}_{kc}")
                        nc.scalar.activation(pt[:, :, :], sv, AFT.Exp,
                                             bias=ebias_sb[:, 0:1],
                                             scale=EXP_ACT_SCALE)
                    else:
                        pt = pt8_pool.tile([128, 2, 512], FP8, tag="pt8",
                                           name=f"pt_{qc}_{h}_{kc}")
                        with nc.allow_low_precision(
                                reason="schraudolph exp to fp8 bits"):
                            nc.vector.tensor_scalar(
                                pt[:, :, :].bitcast(U8), sv,
                                EXP_DVE_MULT, EXP_DVE_BIAS,
                                ALU.mult, ALU.add)

                    def make_pv(kc=kc, pt=pt, pv=pv, h=h, is_bf=want_act):
                        def emit():
                            first = (kc == 0)
                            last = (kc == 15)
                            if is_bf:
                                vb = vtb[kc][:, :, :].rearrange(
                                    "p i (h w) -> p i h w", h=NH)
                                for sub in range(2):
                                    nc.tensor.matmul(
                                        pv[0:65, :],
                                        vb[:, sub, h, 0:65],
                                        pt[:, sub, :],
                                        start=(first and sub == 0),
                                        stop=(last and sub == 1))
                            else:
                                v8 = vt8[kc][:, :, :].rearrange(
                                    "p i (h w) -> p i h w", h=NH)
                                # the hi matmul covers the denominator row
                                # (0:65), so it must carry start on the
                                # first chunk and stop on the last
                                if last:
                                    nc.tensor.matmul(
                                        pv[0:64, :], v8[:, :, h, 72:136],
                                        pt[:, :, :],
                                        start=False, stop=False,
                                        perf_mode=DR)
                                    nc.tensor.matmul(
                                        pv[0:65, :], v8[:, :, h, 0:65],
                                        pt[:, :, :],
                                        start=False, stop=True,
                                        perf_mode=DR)
                                else:
                                    nc.tensor.matmul(
                                        pv[0:65, :], v8[:, :, h, 0:65],
                                        pt[:, :, :],
                                        start=first, stop=False,
                                        perf_mode=DR)
                                    nc.tensor.matmul(
                                        pv[0:64, :], v8[:, :, h, 72:136],
                                        pt[:, :, :],
                                        start=False, stop=False,
                                        perf_mode=DR)
                        return emit

                    pending_pv.append(make_pv())
                    if len(pending_pv) > PV_LAG:
                        pending_pv.pop(0)()
                    if pending_oproj and kc % 4 == 3:
                        pending_oproj.pop(0)()
                for fn in pending_pv:
                    fn()
                # softmax division (the multiply is deferred by one head)
                dn = dn_pool.tile([128, 1, 512], F32, tag="dn",
                                  name=f"dn_{qc}_{h}")
                with nc.allow_low_precision(reason="softmax reciprocal"):
                    nc.vector.reciprocal(dn[64:65, 0, :], pv[64:65, :])
                nc.sync.dma_start(out=dn[0:64, 0, :],
                                  in_=dn[64:65, :, :].to_broadcast([1, 64, 512]))

                def make_div(qc=qc, h=h, pv=pv, dn=dn, qs=qs):
                    def div():
                        cc, odd = h // 2, h % 2
                        with nc.allow_low_precision(
                                reason="fp32r attention output"):
                            if not odd:
                                nc.vector.scalar_tensor_tensor(
                                    outT[cc][0:64, qs], pv[0:64, :], 1.0,
                                    dn[0:64, 0, :], ALU.mult, ALU.mult)
                            else:
                                ot = odd_pool.tile([128, 512], F32R,
                                                   tag="odd",
                                                   name=f"odd_{qc}_{h}")
                                nc.vector.scalar_tensor_tensor(
                                    ot[0:64, :], pv[0:64, :], 1.0,
                                    dn[0:64, 0, :], ALU.mult, ALU.mult)
                                nc.sync.dma_start(out=outT[cc][64:128, qs],
                                                  in_=ot[0:64, :])
                    return div

                if pending_div is not None:
                    pending_div()
                pending_div = make_div()
                if qc == 3 and h >= 2:
                    pending_div()
                    pending_div = None
        if pending_div is not None:
            pending_div()
        for fn in pending_oproj:
            fn()
        for fn in oproj_pieces(3, tail=True):
            fn()


_NC_CACHE = {}


def _get_nc(with_vbias: bool):
    if with_vbias not in _NC_CACHE:
        _NC_CACHE[with_vbias] = build_nc(with_vbias)
    return _NC_CACHE[with_vbias]


def _bf(a):
    return np.asarray(a, dtype=np.float32).astype(NP_BF16)


def make_in_maps(inputs):
    f = {k: np.asarray(v, dtype=np.float32) for k, v in inputs.items()}
    x, context = f["x"], f["context"]
    B = x.shape[0]

    xTs = [_bf(x[b].T) for b in range(B)]
    cTs = [_bf(context[b].T) for b in range(B)]

    in_maps = []
    for b in range(B):
        for hg in range(4):
            sl = slice(hg * CS, (hg + 1) * CS)
            # bqk: per (proj, cc) column; the pair's two 64-channel head
            # halves are stacked along partitions (psum row p of pair cc
            # holds channel 128*cc + p).
            bqk_cols = np.zeros((128, 6), np.float32)
            for j, nm in enumerate(("bq", "bks", "bkc")):
                bb = f[nm][sl]
                for cc in range(2):
                    bqk_cols[:, j * 2 + cc] = bb[cc * 128:(cc + 1) * 128]
            bvm = np.stack([f["bvs"][sl], f["bvc"][sl]]).astype(np.float32)
            in_maps.append({
                "xT": xTs[b],
                "cT": cTs[b],
                "wq": _bf(f["Wq"][:, sl]),
                "wks": _bf(f["Wks"][:, sl]),
                "wkc": _bf(f["Wkc"][:, sl]),
                "wvs": _bf(f["Wvs"][:, sl]),
                "wvc": _bf(f["Wvc"][:, sl]),
                "bqk": np.ascontiguousarray(bqk_cols),
                "bv": np.ascontiguousarray(bvm),
                "wo": np.ascontiguousarray(f["Wo"][sl, :]),
            })
    return in_maps


def kernel(**inputs):
    f32 = {k: np.asarray(v, dtype=np.float32) for k, v in inputs.items()}
    with_vbias = bool(np.any(f32["bvs"]) or np.any(f32["bvc"]))
    nc = _get_nc(with_vbias)

    in_maps = make_in_maps(inputs)
    B = f32["x"].shape[0]

    res = run_bass_kernel_spmd(nc, in_maps, list(range(N_CORES))).results

    bo = f32["bo"]
    out = np.empty((B, T, D), dtype=np.float32)
    for b in range(B):
        acc = res[b * 4 + 0]["out"].astype(np.float32).copy()
        for hg in range(1, 4):
            acc += res[b * 4 + hg]["out"]
        out[b] = acc + bo
    return out
